# revision 20
# baseline (speedup 1.0000x reference)
"""Trainium2 Bass kernel for MultiHeadAttention (B=8, L=1024, D=512, H=8, Qd=64).

Sharding: data-parallel over batch B across the 8 NeuronCores (one batch
element per core).  Per core, for batch element b:

    x_r  = x @ Wc.T + bc                    (pointwise conv)
    Q    = x  @ Wq.T + bq   (per head h: Q_h [L, 64])
    K    = x_r @ Wk.T + bk
    V    = x_r @ Wv.T + bv
    S_h  = Q_h @ K_h.T / 8
    P_h  = softmax(S_h)  -> scores[b, h]    (materialized output)
    A_h  = P_h @ V_h
    out  = concat_h-interleaved(A) @ Wo.T + bo

Layouts (partition dim first):
    XT, XrT, QT, KT : transposed  [D(128-chunks), L]   fp16
    V               : natural     [L(128-chunks), D]   fp16
    S.T psum tiles  : [128 j, 1024 l]  -> exp -> fp16 expST feeds P.T @ V
    S   psum tiles  : [128 l, 1024 j]  -> exp(S/8 - ln(rowsum)) = P -> HBM

All matmul operands are fp16: 1 cyc/row streaming + fast weight load, with
~2^-11 rounding that comfortably fits the value ranges here.  The PE
contracts over the partition dim (out = lhsT.T @ rhs), so the scores matmul
is issued in both orientations (K=64, cheap) instead of transposing P.
Row sums come from a ones-column matmul col-packed next to the P.T @ V
accumulation; the natural-orientation softmax folds 1/rowsum into the exp
as a -ln(rowsum) per-partition bias (rowsum rows are PE-transposed into
per-partition columns).  x and the five weights are transposed via fp16
DMA-transpose through a DRAM bounce rather than on the PE.
"""

from contextlib import ExitStack

import numpy as np

import concourse.bass as bass
import concourse.tile as tile
from concourse import bacc, mybir
from concourse.bass_utils import run_bass_kernel_spmd
from concourse.masks import make_identity

F32 = mybir.dt.float32
F16 = mybir.dt.float16

B, L, D = 8, 1024, 512
H, Qd = 8, 64
NCORES = 8
LC = L // 128   # 8  l-chunks
DC = D // 128   # 4  d/f-chunks
JC = L // 128   # 8  j-chunks
NH = L // 512   # 2  512-wide halves of L

EXPFN = mybir.ActivationFunctionType.Exp


def _bcast_rows(ap, nrows):
    """AP reading a [n]/[1, n] DRAM row as [nrows, n] (0-stride partition).
    Only legal for DRAM sources -- SBUF partition steps must be nonzero."""
    return bass.AP(tensor=ap.tensor, offset=ap.offset,
                   ap=[[1, 1], [0, nrows]] + ap.ap[-1:])


def build_nc():
    nc = bacc.Bacc("TRN2", target_bir_lowering=False, debug=False,
                   num_devices=NCORES)

    x_in = nc.declare_dram_parameter("x", [L, D], F32, isOutput=False)
    w_ins = {
        name: nc.declare_dram_parameter(name, [D, D], F32, isOutput=False)
        for name in ("Wc", "Wq", "Wk", "Wv", "Wo")
    }
    b_ins = {
        name: nc.declare_dram_parameter(name, [D], F32, isOutput=False)
        for name in ("bc", "bq", "bk", "bv", "bo")
    }
    out_out = nc.declare_dram_parameter("out", [L, D], F32, isOutput=True)
    scores_out = nc.declare_dram_parameter("scores", [H, L, L], F32, isOutput=True)

    rs_dram = nc.dram_tensor("rs_bounce", [H, NH, 512], F32)
    x16_dram = nc.dram_tensor("x16_bounce", [L, D], F16)
    w16_dram = {n: nc.dram_tensor(f"{n}16_bounce", [D, D], F16)
                for n in ("Wc", "Wq", "Wk", "Wv", "Wo")}

    with tile.TileContext(nc) as tc:
        early = ExitStack()
        with (
            tc.tile_pool(name="persist", bufs=1) as persist,
            tc.tile_pool(name="ps_a", bufs=2, space="PSUM") as ps_a,
            tc.tile_pool(name="ps_b", bufs=2, space="PSUM") as ps_b,
            tc.tile_pool(name="ps_at", bufs=1, space="PSUM") as ps_at,
        ):
            epool = early.enter_context(tc.tile_pool(name="early", bufs=1))
            stage = early.enter_context(tc.tile_pool(name="stage", bufs=4))

            # ---------------- constants ----------------
            ident = persist.tile([128, 128], F32, name="ident", tag="ident")
            make_identity(nc, ident)
            ones_col = persist.tile([128, 1], F16, name="ones_col", tag="ones_col")
            nc.vector.memset(ones_col, 1.0)

            # per-partition bias chunks: bias[c*128 + p] -> tile[p, c]
            bias_pp = {}
            for name in ("bc", "bq", "bk"):
                t = epool.tile([128, DC], F32, name=f"{name}_pp", tag=f"{name}_pp")
                nc.sync.dma_start(out=t, in_=b_ins[name][:].rearrange("(c p) -> p c", p=128))
                bias_pp[name] = t
            # free-dim broadcast biases
            bias_bc = {}
            for name in ("bv", "bo"):
                t = persist.tile([128, D], F32, name=f"{name}_bc", tag=f"{name}_bc")
                src = b_ins[name][:]
                nc.gpsimd.dma_start(
                    out=t, in_=bass.AP(tensor=src.tensor, offset=src.offset,
                                       ap=[[1, 1], [0, 128]] + src.ap))
                bias_bc[name] = t

            # ------- weights + x: load f32, cast fp16, DMA-transpose -------
            # WT[w][c] = [128 d, 512 f]; XT[c] = [128 d, 1024 l]
            WT = {}
            for wname in ("Wc", "Wq", "Wk", "Wv", "Wo"):
                wpool = persist if wname == "Wo" else epool
                WT[wname] = [
                    wpool.tile([128, D], F16, name=f"{wname}T{c}", tag=f"{wname}T{c}")
                    for c in range(DC)
                ]
            for wname in ("Wc", "Wq", "Wk", "Wv", "Wo"):
                for r in range(DC):
                    w_nat = stage.tile([128, D], F32, name="w_nat", tag="w_nat")
                    nc.sync.dma_start(out=w_nat,
                                      in_=w_ins[wname][r * 128:(r + 1) * 128, :])
                    w16 = stage.tile([128, D], F16, name="w16", tag="w16")
                    nc.vector.tensor_copy(w16, w_nat)
                    nc.sync.dma_start(out=w16_dram[wname][r * 128:(r + 1) * 128, :],
                                      in_=w16)
                for c in range(DC):
                    nc.scalar.dma_start_transpose(
                        WT[wname][c], w16_dram[wname][:, c * 128:(c + 1) * 128])

            XT = [epool.tile([128, L], F16, name=f"XT{c}", tag=f"XT{c}")
                  for c in range(DC)]
            for lc in range(LC):
                x_nat = stage.tile([128, D], F32, name="x_nat", tag="x_nat")
                nc.sync.dma_start(out=x_nat, in_=x_in[lc * 128:(lc + 1) * 128, :])
                x16 = stage.tile([128, D], F16, name="x16", tag="x16")
                nc.vector.tensor_copy(x16, x_nat)
                nc.sync.dma_start(out=x16_dram[lc * 128:(lc + 1) * 128, :], in_=x16)
            for c in range(DC):
                nc.scalar.dma_start_transpose(XT[c], x16_dram[:, c * 128:(c + 1) * 128])

            # ---------------- linear projections ----------------
            XrT = [epool.tile([128, L], F16, name=f"XrT{c}", tag=f"XrT{c}")
                   for c in range(DC)]
            QT = [persist.tile([128, L], F16, name=f"QT{c}", tag=f"QT{c}")
                  for c in range(DC)]
            KT = [persist.tile([128, L], F16, name=f"KT{c}", tag=f"KT{c}")
                  for c in range(DC)]

            for dst, wt, src, bias_t in (
                (XrT, WT["Wc"], XT, bias_pp["bc"]),
                (QT, WT["Wq"], XT, bias_pp["bq"]),
                (KT, WT["Wk"], XrT, bias_pp["bk"]),
            ):
                for fc in range(DC):
                    # lh0/lh1 share each stationary weight block back-to-back
                    ps0 = ps_a.tile([128, 512], F32, name="lps0", tag="ps_a")
                    ps1 = ps_b.tile([128, 512], F32, name="lps1", tag="ps_b")
                    for dc in range(DC):
                        wblk = wt[dc][:, fc * 128:(fc + 1) * 128]
                        nc.tensor.matmul(ps0, wblk, src[dc][:, 0:512],
                                         start=(dc == 0), stop=(dc == DC - 1))
                        nc.tensor.matmul(ps1, wblk, src[dc][:, 512:1024],
                                         start=(dc == 0), stop=(dc == DC - 1))
                    nc.vector.tensor_scalar_add(dst[fc][:, 0:512], ps0,
                                                bias_t[:, fc:fc + 1])
                    nc.vector.tensor_scalar_add(dst[fc][:, 512:1024], ps1,
                                                bias_t[:, fc:fc + 1])

            # V natural: V[jc] = [128 j, 512 f]
            V = [persist.tile([128, D], F16, name=f"V{jc}", tag=f"V{jc}")
                 for jc in range(JC)]
            for jc in range(JC):
                ps = ps_a.tile([128, 512], F32, name="lps0", tag="ps_a")
                for dc in range(DC):
                    nc.tensor.matmul(ps, XrT[dc][:, jc * 128:(jc + 1) * 128],
                                     WT["Wv"][dc],
                                     start=(dc == 0), stop=(dc == DC - 1))
                nc.vector.tensor_add(V[jc], ps, bias_bc["bv"])

            # ---------------- attention (per head pair) ----------------
            early.close()  # reclaim XT/XrT/W-stage SBUF
            late = ExitStack()
            expst_pool = late.enter_context(tc.tile_pool(name="expst", bufs=2))
            pnat_pool = late.enter_context(tc.tile_pool(name="pnat", bufs=3))
            small = late.enter_context(tc.tile_pool(name="small", bufs=2))
            osb_pool = late.enter_context(tc.tile_pool(name="osb", bufs=2))
            mergedT = [persist.tile([128, L], F16, name=f"mgT{mc}", tag=f"mgT{mc}")
                       for mc in range(DC)]

            for pair in range(H // 2):
                hA, hB = 2 * pair, 2 * pair + 1
                ft = pair  # QT/KT chunk; hA at partitions 0:64, hB at 64:128
                qtA, ktA = QT[ft][0:64, :], KT[ft][0:64, :]
                qtB, ktB = QT[ft][64:128, :], KT[ft][64:128, :]

                # transposed path: S.T -> exp -> (P.T @ V, ones row sums)
                at_ps = ps_at.tile([128, L], F32, name="at_ps", tag="at_ps")
                rs_ps = ps_a.tile([128, 512], F32, name="rs_ps", tag="ps_a")
                rs_pos = {(hA, 0): 0, (hA, 1): 32, (hB, 0): 64, (hB, 1): 96}

                for jc in range(JC):
                    stA = ps_b.tile([128, L], F32, name="st_ps", tag="ps_b")
                    stB = ps_b.tile([128, L], F32, name="st_ps", tag="ps_b")
                    for lh in range(NH):
                        sl = slice(lh * 512, (lh + 1) * 512)
                        nc.tensor.matmul(stA[:, sl], ktA[:, jc * 128:(jc + 1) * 128],
                                         qtA[:, sl], start=True, stop=True)
                        nc.tensor.matmul(stB[:, sl], ktB[:, jc * 128:(jc + 1) * 128],
                                         qtB[:, sl], start=True, stop=True)
                    eA = expst_pool.tile([128, L], F16, name="expstA", tag="expstA")
                    eB = expst_pool.tile([128, L], F16, name="expstB", tag="expstB")
                    nc.scalar.activation(eA, stA, EXPFN, scale=0.125)
                    nc.scalar.activation(eB, stB, EXPFN, scale=0.125)
                    first, last = (jc == 0), (jc == JC - 1)
                    for head, e, base in ((hA, eA, 0), (hB, eB, 64)):
                        for lh in range(NH):
                            sl = slice(lh * 512, (lh + 1) * 512)
                            nc.tensor.matmul(
                                at_ps[base:base + 64, sl],
                                V[jc][:, head * 64:(head + 1) * 64], e[:, sl],
                                start=first, stop=last)
                        for lh in range(NH):
                            sl = slice(lh * 512, (lh + 1) * 512)
                            p = rs_pos[(head, lh)]
                            nc.tensor.matmul(
                                rs_ps[p:p + 1, :], ones_col, e[:, sl],
                                start=first, stop=last, tile_position=(0, p))

                # row sums: rows -> per-partition columns (PE transpose) for
                # the natural path, and -> DRAM -> broadcast rows for the
                # P.T @ V rescale.
                rs_sb = small.tile([128, 512], F32, name="rs_sb", tag="rs_sb")
                nc.vector.tensor_copy(rs_sb, rs_ps)  # rows 0/32/64/96 live
                rs_natT = small.tile([128, 512], F32, name="rs_natT", tag="rs_natT")
                for c in range(DC):
                    tps = ps_a.tile([128, 128], F32, name="tps", tag="ps_a")
                    nc.tensor.transpose(tps, rs_sb[:, c * 128:(c + 1) * 128], ident)
                    nc.vector.tensor_copy(rs_natT[:, c * 128:(c + 1) * 128], tps)
                negln = small.tile([128, 512], F32, name="negln", tag="negln")
                nc.scalar.activation(negln, rs_natT,
                                     mybir.ActivationFunctionType.Ln)
                nc.vector.tensor_scalar_mul(negln, negln, -1.0)

                def negln_col(head, lc, _hA=hA, _negln=negln):
                    col = (lc % 4) * 128 + (0 if head == _hA else 64) + 32 * (lc // 4)
                    return _negln[:, col:col + 1]

                # natural path: P = exp(S/8 - ln(rowsum)) -> scores
                for lc in range(LC):
                    for head, qt, kt in ((hA, qtA, ktA), (hB, qtB, ktB)):
                        nat = ps_b.tile([128, L], F32, name="nat_ps", tag="ps_b")
                        for lh in range(NH):
                            sl = slice(lh * 512, (lh + 1) * 512)
                            nc.tensor.matmul(nat[:, sl],
                                             qt[:, lc * 128:(lc + 1) * 128],
                                             kt[:, sl], start=True, stop=True)
                        pn = pnat_pool.tile([128, L], F32, name="pnat", tag="pnat")
                        nc.scalar.activation(pn, nat, EXPFN, scale=0.125,
                                             bias=negln_col(head, lc))
                        nc.gpsimd.dma_start(
                            out=scores_out[head, lc * 128:(lc + 1) * 128, :], in_=pn)

                # rescale P.T @ V by 1/rowsum (broadcast along l) and merge
                # into the (d, h)-interleaved layout for the out projection.
                rsb = small.tile([128, L], F32, name="rsb", tag="rsb")
                for head, hbase in ((hA, 0), (hB, 64)):
                    for lh in range(NH):
                        p = hbase + 32 * lh
                        nc.sync.dma_start(out=rs_dram[head, lh, :],
                                          in_=rs_sb[p:p + 1, :])
                        nc.gpsimd.dma_start(
                            out=rsb[hbase:hbase + 64, lh * 512:(lh + 1) * 512],
                            in_=_bcast_rows(rs_dram[head, lh, :], 64))
                rcb = small.tile([128, L], F32, name="rcb", tag="rcb")
                nc.vector.reciprocal_approx_fast(rcb, rsb)
                asb = small.tile([128, L], F16, name="asb", tag="asb")
                nc.vector.tensor_mul(asb, at_ps, rcb)
                for head, base in ((hA, 0), (hB, 64)):
                    for mc in range(DC):
                        nc.gpsimd.dma_start(
                            out=mergedT[mc][head::8, :],
                            in_=asb[base + 16 * mc:base + 16 * mc + 16, :])

            # ---------------- output projection ----------------
            for lc in range(LC):
                ps = ps_a.tile([128, 512], F32, name="lps0", tag="ps_a")
                for mc in range(DC):
                    nc.tensor.matmul(ps, mergedT[mc][:, lc * 128:(lc + 1) * 128],
                                     WT["Wo"][mc], start=(mc == 0),
                                     stop=(mc == DC - 1))
                osb = osb_pool.tile([128, D], F32, name="osb", tag="osb")
                nc.vector.tensor_add(osb, ps, bias_bc["bo"])
                nc.sync.dma_start(out=out_out[lc * 128:(lc + 1) * 128, :], in_=osb)
            late.close()

    nc.compile()
    return nc


_NC_CACHE = None


def _get_nc():
    global _NC_CACHE
    if _NC_CACHE is None:
        _NC_CACHE = build_nc()
    return _NC_CACHE


def run(inputs, trace=False):
    """Run on 8 cores; returns (out, scores, BassKernelResults)."""
    nc = _get_nc()
    core_ids = list(range(NCORES))
    x = np.ascontiguousarray(np.asarray(inputs["x"], dtype=np.float32))
    shared = {}
    for name in ("Wc", "Wq", "Wk", "Wv", "Wo", "bc", "bq", "bk", "bv", "bo"):
        shared[name] = np.ascontiguousarray(np.asarray(inputs[name], dtype=np.float32))
    in_maps = [dict(shared, x=x[b]) for b in core_ids]
    res = run_bass_kernel_spmd(nc, in_maps, core_ids, trace=trace)
    out = np.stack([res.results[b]["out"] for b in core_ids])
    scores = np.stack([res.results[b]["scores"] for b in core_ids])
    return out, scores, res


def kernel(**inputs):
    out, scores, _ = run(inputs)
    return out, scores


# revision 21
# speedup vs baseline: 1.2767x; 1.2767x over previous
"""Trainium2 Bass kernel for MultiHeadAttention (B=8, L=1024, D=512, H=8, Qd=64).

Sharding: data-parallel over batch B across the 8 NeuronCores (one batch
element per core).  Per core, for batch element b:

    x_r  = x @ Wc.T + bc                    (pointwise conv)
    Q    = x  @ Wq.T + bq   (per head h: Q_h [L, 64])
    K    = x_r @ Wk.T + bk
    V    = x_r @ Wv.T + bv
    S_h  = Q_h @ K_h.T / 8
    P_h  = softmax(S_h)  -> scores[b, h]    (materialized output)
    A_h  = P_h @ V_h
    out  = concat_h-interleaved(A) @ Wo.T + bo

Layouts (partition dim first):
    XT, XrT, QT, KT : transposed  [D(128-chunks), L]   fp16
    V               : natural     [L(128-chunks), D]   fp16
    S   psum tiles  : [128 l, 1024 j]  -> exp (+row-sum accum) -> P -> HBM
    S.T psum tiles  : [128 j, 1024 l]  -> exp -> fp16 expST feeds P.T @ V

All matmul operands are fp16 (1 cyc/row streaming + fast weight load; the
~2^-11 rounding comfortably fits the value ranges here).  The PE contracts
over the partition dim (out = lhsT.T @ rhs), so the scores matmul runs in
both orientations (K=64, cheap) instead of transposing P on chip.

Phase order maximizes ScalarE (exp) density, the true bottleneck:
natural-orientation softmax for ALL heads first (rowsums fall out of the
activation accumulator), then the transposed path for all heads (attention
accumulation); the natural rowsum columns are PE-transposed into row form
and broadcast (via a DRAM bounce) to rescale the P.T @ V output.  x and
the weights are fp16-transposed via DMA-transpose through a contiguous
DRAM bounce rather than on the PE.
"""

from contextlib import ExitStack

import numpy as np

import concourse.bass as bass
import concourse.tile as tile
from concourse import bacc, mybir
from concourse.bass_utils import run_bass_kernel_spmd
from concourse.masks import make_identity

F32 = mybir.dt.float32
F16 = mybir.dt.float16

B, L, D = 8, 1024, 512
H, Qd = 8, 64
NCORES = 8
LC = L // 128   # 8  l-chunks
DC = D // 128   # 4  d/f-chunks
JC = L // 128   # 8  j-chunks
NH = L // 512   # 2  512-wide halves of L

EXPFN = mybir.ActivationFunctionType.Exp


def _bcast_rows(ap, nrows):
    """AP reading a [n] DRAM row as [nrows, n] (0-stride partition dim).
    Only legal for DRAM sources -- SBUF partition steps must be nonzero."""
    return bass.AP(tensor=ap.tensor, offset=ap.offset,
                   ap=[[1, 1], [0, nrows]] + ap.ap[-1:])


def build_nc():
    nc = bacc.Bacc("TRN2", target_bir_lowering=False, debug=False,
                   num_devices=NCORES)

    x_in = nc.declare_dram_parameter("x", [L, D], F32, isOutput=False)
    w_ins = {
        name: nc.declare_dram_parameter(name, [D, D], F32, isOutput=False)
        for name in ("Wc", "Wq", "Wk", "Wv", "Wo")
    }
    b_ins = {
        name: nc.declare_dram_parameter(name, [D], F32, isOutput=False)
        for name in ("bc", "bq", "bk", "bv", "bo")
    }
    out_out = nc.declare_dram_parameter("out", [L, D], F32, isOutput=True)
    scores_out = nc.declare_dram_parameter("scores", [H, L, L], F32, isOutput=True)

    rs_dram = nc.dram_tensor("rs_bounce", [H, L], F32)
    # fp16 bounce buffers in chunk-major layout so each DMA-transpose input
    # [rows, 128] is one contiguous block.
    x16c = nc.dram_tensor("x16c_bounce", [DC, L, 128], F16)
    w16c = {n: nc.dram_tensor(f"{n}16c_bounce", [DC, D, 128], F16)
            for n in ("Wc", "Wq", "Wk", "Wv", "Wo")}

    with tile.TileContext(nc) as tc:
        early = ExitStack()
        with (
            tc.tile_pool(name="persist", bufs=1) as persist,
            tc.tile_pool(name="ps_a", bufs=2, space="PSUM") as ps_a,
            tc.tile_pool(name="ps_big", bufs=2, space="PSUM") as ps_big,
            tc.tile_pool(name="ps_at", bufs=1, space="PSUM") as ps_at,
        ):
            epool = early.enter_context(tc.tile_pool(name="early", bufs=1))
            stage = early.enter_context(tc.tile_pool(name="stage", bufs=4))

            # ---------------- constants ----------------
            ident = persist.tile([128, 128], F32, name="ident", tag="ident")
            make_identity(nc, ident)

            bias_pp = {}
            for name in ("bc", "bq", "bk"):
                t = epool.tile([128, DC], F32, name=f"{name}_pp", tag=f"{name}_pp")
                nc.sync.dma_start(out=t, in_=b_ins[name][:].rearrange("(c p) -> p c", p=128))
                bias_pp[name] = t
            bias_bc = {}
            for name in ("bv", "bo"):
                t = persist.tile([128, D], F32, name=f"{name}_bc", tag=f"{name}_bc")
                src = b_ins[name][:]
                nc.gpsimd.dma_start(
                    out=t, in_=bass.AP(tensor=src.tensor, offset=src.offset,
                                       ap=[[1, 1], [0, 128]] + src.ap))
                bias_bc[name] = t

            # ------- weights + x: load f32, cast fp16, DMA-transpose -------
            # WT[w][c] = [128 d, 512 f]; XT[c] = [128 d, 1024 l]
            WT = {}
            for wname in ("Wc", "Wq", "Wk", "Wv", "Wo"):
                wpool = persist if wname in ("Wv", "Wo") else epool
                WT[wname] = [
                    wpool.tile([128, D], F16, name=f"{wname}T{c}", tag=f"{wname}T{c}")
                    for c in range(DC)
                ]

            def prep_weight(wname):
                for r in range(DC):
                    w_nat = stage.tile([128, D], F32, name="w_nat", tag="w_nat")
                    nc.sync.dma_start(out=w_nat,
                                      in_=w_ins[wname][r * 128:(r + 1) * 128, :])
                    w16 = stage.tile([128, D], F16, name="w16", tag="w16")
                    nc.vector.tensor_copy(w16, w_nat)
                    dst = w16c[wname][:, r * 128:(r + 1) * 128, :]
                    nc.sync.dma_start(out=dst.rearrange("c p w -> p c w"), in_=w16)
                for c in range(DC):
                    nc.scalar.dma_start_transpose(WT[wname][c], w16c[wname][c])

            XT = [epool.tile([128, L], F16, name=f"XT{c}", tag=f"XT{c}")
                  for c in range(DC)]

            def prep_x():
                for lc in range(LC):
                    x_nat = stage.tile([128, D], F32, name="x_nat", tag="x_nat")
                    nc.sync.dma_start(out=x_nat, in_=x_in[lc * 128:(lc + 1) * 128, :])
                    x16 = stage.tile([128, D], F16, name="x16", tag="x16")
                    nc.vector.tensor_copy(x16, x_nat)
                    dst = x16c[:, lc * 128:(lc + 1) * 128, :]
                    nc.sync.dma_start(out=dst.rearrange("c p w -> p c w"), in_=x16)
                for c in range(DC):
                    nc.scalar.dma_start_transpose(XT[c], x16c[c])

            prep_x()
            for wname in ("Wc", "Wq", "Wk", "Wv", "Wo"):
                prep_weight(wname)

            # ---------------- linear projections ----------------
            XrT = [epool.tile([128, L], F16, name=f"XrT{c}", tag=f"XrT{c}")
                   for c in range(DC)]
            QT = [persist.tile([128, L], F16, name=f"QT{c}", tag=f"QT{c}")
                  for c in range(DC)]
            KT = [persist.tile([128, L], F16, name=f"KT{c}", tag=f"KT{c}")
                  for c in range(DC)]

            for dst, wt, src, bias_t in (
                (XrT, WT["Wc"], XT, bias_pp["bc"]),
                (QT, WT["Wq"], XT, bias_pp["bq"]),
                (KT, WT["Wk"], XrT, bias_pp["bk"]),
            ):
                for fc in range(DC):
                    # lh0/lh1 share each stationary weight block back-to-back
                    ps0 = ps_a.tile([128, 512], F32, name="lps0", tag="ps_a")
                    ps1 = ps_big.tile([128, 512], F32, name="lps1", tag="ps_big")
                    for dc in range(DC):
                        wblk = wt[dc][:, fc * 128:(fc + 1) * 128]
                        nc.tensor.matmul(ps0, wblk, src[dc][:, 0:512],
                                         start=(dc == 0), stop=(dc == DC - 1))
                        nc.tensor.matmul(ps1, wblk, src[dc][:, 512:1024],
                                         start=(dc == 0), stop=(dc == DC - 1))
                    nc.vector.tensor_scalar_add(dst[fc][:, 0:512], ps0,
                                                bias_t[:, fc:fc + 1])
                    nc.vector.tensor_scalar_add(dst[fc][:, 512:1024], ps1,
                                                bias_t[:, fc:fc + 1])

            # V natural: V[jc] = [128 j, 512 f]
            V = [persist.tile([128, D], F16, name=f"V{jc}", tag=f"V{jc}")
                 for jc in range(JC)]
            for jc in range(JC):
                ps = ps_a.tile([128, 512], F32, name="lps0", tag="ps_a")
                for dc in range(DC):
                    nc.tensor.matmul(ps, XrT[dc][:, jc * 128:(jc + 1) * 128],
                                     WT["Wv"][dc],
                                     start=(dc == 0), stop=(dc == DC - 1))
                nc.vector.tensor_add(V[jc], ps, bias_bc["bv"])

            early.close()  # reclaim XT/XrT/WcT/WqT/WkT/stage SBUF
            late = ExitStack()
            expst_pool = late.enter_context(tc.tile_pool(name="expst", bufs=2))
            pnat_pool = late.enter_context(tc.tile_pool(name="pnat", bufs=4))
            small = late.enter_context(tc.tile_pool(name="small", bufs=2))
            rsp_pool = late.enter_context(tc.tile_pool(name="rsp", bufs=1))
            osb_pool = late.enter_context(tc.tile_pool(name="osb", bufs=2))
            mergedT = [persist.tile([128, L], F16, name=f"mgT{mc}", tag=f"mgT{mc}")
                       for mc in range(DC)]

            def qtkt(head):
                ft, po = head // 2, (head % 2) * 64
                return (QT[ft][po:po + 64, :], KT[ft][po:po + 64, :])

            # -------- phase N: natural softmax, all heads -> scores --------
            # Row sums land in per-head [128, LC] column packs via the
            # activation accumulator.
            rs_pack = [rsp_pool.tile([128, LC], F32, name=f"rsp{h}", tag=f"rsp{h}")
                       for h in range(H)]
            for pair in range(H // 2):
                hA, hB = 2 * pair, 2 * pair + 1
                for lc in range(LC):
                    for head in (hA, hB):
                        qt, kt = qtkt(head)
                        nat = ps_big.tile([128, L], F32, name="nat_ps", tag="ps_big")
                        for lh in range(NH):
                            sl = slice(lh * 512, (lh + 1) * 512)
                            nc.tensor.matmul(nat[:, sl],
                                             qt[:, lc * 128:(lc + 1) * 128],
                                             kt[:, sl], start=True, stop=True)
                        pn = pnat_pool.tile([128, L], F32, name="pnat", tag="pnat")
                        rs_col = rs_pack[head][:, lc:lc + 1]
                        nc.scalar.activation(pn, nat, EXPFN, scale=0.125,
                                             accum_out=rs_col)
                        rc_n = small.tile([128, 1], F32, name="rc_n", tag="rc_n",
                                          bufs=6)
                        nc.vector.reciprocal(rc_n, rs_col)
                        nc.vector.tensor_scalar_mul(pn, pn, rc_n)
                        nc.sync.dma_start(
                            out=scores_out[head, lc * 128:(lc + 1) * 128, :], in_=pn)

            # ---- phase T: transposed path (P.T @ V), per head pair ----
            for pair in range(H // 2):
                hA, hB = 2 * pair, 2 * pair + 1
                qtA, ktA = qtkt(hA)
                qtB, ktB = qtkt(hB)

                at_ps = ps_at.tile([128, L], F32, name="at_ps", tag="at_ps")
                for jc in range(JC):
                    stA = ps_big.tile([128, L], F32, name="st_ps", tag="ps_big")
                    stB = ps_big.tile([128, L], F32, name="st_ps", tag="ps_big")
                    for lh in range(NH):
                        sl = slice(lh * 512, (lh + 1) * 512)
                        nc.tensor.matmul(stA[:, sl], ktA[:, jc * 128:(jc + 1) * 128],
                                         qtA[:, sl], start=True, stop=True)
                        nc.tensor.matmul(stB[:, sl], ktB[:, jc * 128:(jc + 1) * 128],
                                         qtB[:, sl], start=True, stop=True)
                    eA = expst_pool.tile([128, L], F16, name="expstA", tag="expstA")
                    eB = expst_pool.tile([128, L], F16, name="expstB", tag="expstB")
                    nc.scalar.activation(eA, stA, EXPFN, scale=0.125)
                    nc.scalar.activation(eB, stB, EXPFN, scale=0.125)
                    first, last = (jc == 0), (jc == JC - 1)
                    for lh in range(NH):
                        sl = slice(lh * 512, (lh + 1) * 512)
                        nc.tensor.matmul(at_ps[0:64, sl],
                                         V[jc][:, hA * 64:(hA + 1) * 64], eA[:, sl],
                                         start=first, stop=last)
                        nc.tensor.matmul(at_ps[64:128, sl],
                                         V[jc][:, hB * 64:(hB + 1) * 64], eB[:, sl],
                                         start=first, stop=last)

                # free at_ps immediately; rescale later off the PE path
                araw = small.tile([128, L], F32, name="araw", tag="araw")
                nc.vector.tensor_copy(araw, at_ps)

                # natural-phase rowsum columns -> row form -> DRAM -> bcast
                rsb = small.tile([128, L], F32, name="rsb", tag="rsb")
                for head, hbase in ((hA, 0), (hB, 64)):
                    tps = ps_a.tile([128, 128], F32, name="tps", tag="ps_a")
                    nc.tensor.transpose(tps[0:LC, :], rs_pack[head], ident)
                    rsr = small.tile([LC, 128], F32, name="rsr", tag="rsr", bufs=4)
                    nc.vector.tensor_copy(rsr, tps[0:LC, :])
                    nc.sync.dma_start(out=rs_dram[head, :], in_=rsr)
                    nc.gpsimd.dma_start(out=rsb[hbase:hbase + 64, :],
                                        in_=_bcast_rows(rs_dram[head, :], 64))
                rcb = small.tile([128, L], F32, name="rcb", tag="rcb")
                nc.vector.reciprocal_approx_fast(rcb, rsb)
                asb = small.tile([128, L], F16, name="asb", tag="asb")
                nc.vector.tensor_mul(asb, araw, rcb)
                for head, base in ((hA, 0), (hB, 64)):
                    for mc in range(DC):
                        nc.gpsimd.dma_start(
                            out=mergedT[mc][head::8, :],
                            in_=asb[base + 16 * mc:base + 16 * mc + 16, :])

            # ---------------- output projection ----------------
            for lc in range(LC):
                ps = ps_a.tile([128, 512], F32, name="lps0", tag="ps_a")
                for mc in range(DC):
                    nc.tensor.matmul(ps, mergedT[mc][:, lc * 128:(lc + 1) * 128],
                                     WT["Wo"][mc], start=(mc == 0),
                                     stop=(mc == DC - 1))
                osb = osb_pool.tile([128, D], F32, name="osb", tag="osb")
                nc.vector.tensor_add(osb, ps, bias_bc["bo"])
                nc.sync.dma_start(out=out_out[lc * 128:(lc + 1) * 128, :], in_=osb)
            late.close()

    nc.compile()
    return nc


_NC_CACHE = None


def _get_nc():
    global _NC_CACHE
    if _NC_CACHE is None:
        _NC_CACHE = build_nc()
    return _NC_CACHE


def run(inputs, trace=False):
    """Run on 8 cores; returns (out, scores, BassKernelResults)."""
    nc = _get_nc()
    core_ids = list(range(NCORES))
    x = np.ascontiguousarray(np.asarray(inputs["x"], dtype=np.float32))
    shared = {}
    for name in ("Wc", "Wq", "Wk", "Wv", "Wo", "bc", "bq", "bk", "bv", "bo"):
        shared[name] = np.ascontiguousarray(np.asarray(inputs[name], dtype=np.float32))
    in_maps = [dict(shared, x=x[b]) for b in core_ids]
    res = run_bass_kernel_spmd(nc, in_maps, core_ids, trace=trace)
    out = np.stack([res.results[b]["out"] for b in core_ids])
    scores = np.stack([res.results[b]["scores"] for b in core_ids])
    return out, scores, res


def kernel(**inputs):
    out, scores, _ = run(inputs)
    return out, scores


# revision 22
# speedup vs baseline: 1.4999x; 1.1749x over previous
"""Trainium2 Bass kernel for MultiHeadAttention (B=8, L=1024, D=512, H=8, Qd=64).

Sharding: data-parallel over batch B across the 8 NeuronCores (one batch
element per core).  Per core, for batch element b:

    x_r  = x @ Wc.T + bc                    (pointwise conv)
    Q    = x  @ Wq.T + bq   (per head h: Q_h [L, 64])
    K    = x_r @ Wk.T + bk
    V    = x_r @ Wv.T + bv
    S_h  = Q_h @ K_h.T / 8
    P_h  = softmax(S_h)  -> scores[b, h]    (materialized output)
    A_h  = P_h @ V_h
    out  = concat_h-interleaved(A) @ Wo.T + bo

Layouts (partition dim first):
    XT, XrT, QT, KT : transposed  [D(128-chunks), L]   fp16
    V               : natural     [L(128-chunks), D]   fp16
    S   psum tiles  : [128 l, 1024 j]  -> exp (+row-sum accum) -> P -> HBM
    S.T psum tiles  : [128 j, 1024 l]  -> exp -> fp16 expST feeds P.T @ V

All matmul operands are fp16 (1 cyc/row streaming + fast weight load; the
~2^-11 rounding comfortably fits the value ranges here).  The PE contracts
over the partition dim (out = lhsT.T @ rhs), so the scores matmul runs in
both orientations (K=64, cheap) instead of transposing P on chip.

Phase order maximizes ScalarE (exp) density, the true bottleneck:
natural-orientation softmax for ALL heads first (rowsums fall out of the
activation accumulator), then the transposed path for all heads (attention
accumulation); the natural rowsum columns are PE-transposed into row form
and broadcast (via a DRAM bounce) to rescale the P.T @ V output.  x and
the weights are fp16-transposed via DMA-transpose through a contiguous
DRAM bounce rather than on the PE.
"""

from contextlib import ExitStack

import numpy as np

import concourse.bass as bass
import concourse.tile as tile
from concourse import bacc, mybir
from concourse.bass_utils import run_bass_kernel_spmd
from concourse.masks import make_identity

F32 = mybir.dt.float32
F16 = mybir.dt.float16

B, L, D = 8, 1024, 512
H, Qd = 8, 64
NCORES = 8
LC = L // 128   # 8  l-chunks
DC = D // 128   # 4  d/f-chunks
JC = L // 128   # 8  j-chunks
NH = L // 512   # 2  512-wide halves of L

EXPFN = mybir.ActivationFunctionType.Exp


def _bcast_rows(ap, nrows):
    """AP reading a [n] DRAM row as [nrows, n] (0-stride partition dim).
    Only legal for DRAM sources -- SBUF partition steps must be nonzero."""
    return bass.AP(tensor=ap.tensor, offset=ap.offset,
                   ap=[[1, 1], [0, nrows]] + ap.ap[-1:])


def build_nc():
    nc = bacc.Bacc("TRN2", target_bir_lowering=False, debug=False,
                   num_devices=NCORES)

    x_in = nc.declare_dram_parameter("x", [L, D], F32, isOutput=False)
    w_ins = {
        name: nc.declare_dram_parameter(name, [D, D], F32, isOutput=False)
        for name in ("Wc", "Wq", "Wk", "Wv", "Wo")
    }
    b_ins = {
        name: nc.declare_dram_parameter(name, [D], F32, isOutput=False)
        for name in ("bc", "bq", "bk", "bv", "bo")
    }
    out_out = nc.declare_dram_parameter("out", [L, D], F32, isOutput=True)
    scores_out = nc.declare_dram_parameter("scores", [H, L, L], F32, isOutput=True)

    rs_dram = nc.dram_tensor("rs_bounce", [H, L], F32)

    with tile.TileContext(nc) as tc:
        early = ExitStack()
        with (
            tc.tile_pool(name="persist", bufs=1) as persist,
            tc.tile_pool(name="ps_a", bufs=2, space="PSUM") as ps_a,
            tc.tile_pool(name="ps_big", bufs=2, space="PSUM") as ps_big,
            tc.tile_pool(name="ps_at", bufs=1, space="PSUM") as ps_at,
        ):
            epool = early.enter_context(tc.tile_pool(name="early", bufs=1))
            stage = early.enter_context(tc.tile_pool(name="stage", bufs=4))

            # ---------------- constants ----------------
            ident = persist.tile([128, 128], F32, name="ident", tag="ident")
            make_identity(nc, ident)

            bias_pp = {}
            for name in ("bc", "bq", "bk"):
                t = epool.tile([128, DC], F32, name=f"{name}_pp", tag=f"{name}_pp")
                nc.sync.dma_start(out=t, in_=b_ins[name][:].rearrange("(c p) -> p c", p=128))
                bias_pp[name] = t
            bias_bc = {}
            for name in ("bv", "bo"):
                t = persist.tile([128, D], F32, name=f"{name}_bc", tag=f"{name}_bc")
                src = b_ins[name][:]
                nc.gpsimd.dma_start(
                    out=t, in_=bass.AP(tensor=src.tensor, offset=src.offset,
                                       ap=[[1, 1], [0, 128]] + src.ap))
                bias_bc[name] = t

            # ------- weights + x: load f32, cast fp16, DMA-transpose -------
            # WT[w][c] = [128 d, 512 f]; XT[c] = [128 d, 1024 l]
            WT = {}
            for wname in ("Wc", "Wq", "Wk", "Wv", "Wo"):
                wpool = persist if wname in ("Wv", "Wo") else epool
                WT[wname] = [
                    wpool.tile([128, D], F16, name=f"{wname}T{c}", tag=f"{wname}T{c}")
                    for c in range(DC)
                ]

            # PE-transpose [128,128] blocks (f32), casting to fp16 on the
            # PSUM->SBUF copy.  Alternating psum pools keep 3 blocks in
            # flight; DMA-transpose is avoided (each one drains the DMA
            # engines for ~5us on this toolchain).
            tp_n = [0]

            def pe_transpose_into(dst_tile, dst_cols, src_slice):
                pool = (ps_a, ps_at)[tp_n[0] % 2]
                tag = ("ps_a", "at_ps")[tp_n[0] % 2]
                tp_n[0] += 1
                ps = pool.tile([128, 128], F32, name="tps", tag=tag)
                nc.tensor.transpose(ps, src_slice, ident)
                nc.vector.tensor_copy(dst_tile[:, dst_cols], ps)

            def prep_weight(wname):
                for r in range(DC):
                    w_nat = stage.tile([128, D], F32, name="w_nat", tag="w_nat")
                    nc.sync.dma_start(out=w_nat,
                                      in_=w_ins[wname][r * 128:(r + 1) * 128, :])
                    for c in range(DC):
                        pe_transpose_into(WT[wname][c],
                                          slice(r * 128, (r + 1) * 128),
                                          w_nat[:, c * 128:(c + 1) * 128])

            XT = [epool.tile([128, L], F16, name=f"XT{c}", tag=f"XT{c}")
                  for c in range(DC)]

            def prep_x():
                for lc in range(LC):
                    x_nat = stage.tile([128, D], F32, name="x_nat", tag="x_nat")
                    nc.sync.dma_start(out=x_nat, in_=x_in[lc * 128:(lc + 1) * 128, :])
                    for c in range(DC):
                        pe_transpose_into(XT[c], slice(lc * 128, (lc + 1) * 128),
                                          x_nat[:, c * 128:(c + 1) * 128])

            prep_x()
            for wname in ("Wc", "Wq", "Wk", "Wv", "Wo"):
                prep_weight(wname)

            # ---------------- linear projections ----------------
            XrT = [epool.tile([128, L], F16, name=f"XrT{c}", tag=f"XrT{c}")
                   for c in range(DC)]
            QT = [persist.tile([128, L], F16, name=f"QT{c}", tag=f"QT{c}")
                  for c in range(DC)]
            KT = [persist.tile([128, L], F16, name=f"KT{c}", tag=f"KT{c}")
                  for c in range(DC)]

            for dst, wt, src, bias_t in (
                (XrT, WT["Wc"], XT, bias_pp["bc"]),
                (QT, WT["Wq"], XT, bias_pp["bq"]),
                (KT, WT["Wk"], XrT, bias_pp["bk"]),
            ):
                for fc in range(DC):
                    # lh0/lh1 share each stationary weight block back-to-back
                    ps0 = ps_a.tile([128, 512], F32, name="lps0", tag="ps_a")
                    ps1 = ps_big.tile([128, 512], F32, name="lps1", tag="ps_big")
                    for dc in range(DC):
                        wblk = wt[dc][:, fc * 128:(fc + 1) * 128]
                        nc.tensor.matmul(ps0, wblk, src[dc][:, 0:512],
                                         start=(dc == 0), stop=(dc == DC - 1))
                        nc.tensor.matmul(ps1, wblk, src[dc][:, 512:1024],
                                         start=(dc == 0), stop=(dc == DC - 1))
                    nc.vector.tensor_scalar_add(dst[fc][:, 0:512], ps0,
                                                bias_t[:, fc:fc + 1])
                    nc.vector.tensor_scalar_add(dst[fc][:, 512:1024], ps1,
                                                bias_t[:, fc:fc + 1])

            # V natural: V[jc] = [128 j, 512 f]
            V = [persist.tile([128, D], F16, name=f"V{jc}", tag=f"V{jc}")
                 for jc in range(JC)]
            for jc in range(JC):
                ps = ps_a.tile([128, 512], F32, name="lps0", tag="ps_a")
                for dc in range(DC):
                    nc.tensor.matmul(ps, XrT[dc][:, jc * 128:(jc + 1) * 128],
                                     WT["Wv"][dc],
                                     start=(dc == 0), stop=(dc == DC - 1))
                nc.vector.tensor_add(V[jc], ps, bias_bc["bv"])

            early.close()  # reclaim XT/XrT/WcT/WqT/WkT/stage SBUF
            late = ExitStack()
            expst_pool = late.enter_context(tc.tile_pool(name="expst", bufs=2))
            pnat_pool = late.enter_context(tc.tile_pool(name="pnat", bufs=4))
            small = late.enter_context(tc.tile_pool(name="small", bufs=2))
            rsp_pool = late.enter_context(tc.tile_pool(name="rsp", bufs=1))
            osb_pool = late.enter_context(tc.tile_pool(name="osb", bufs=2))
            mergedT = [persist.tile([128, L], F16, name=f"mgT{mc}", tag=f"mgT{mc}")
                       for mc in range(DC)]

            def qtkt(head):
                ft, po = head // 2, (head % 2) * 64
                return (QT[ft][po:po + 64, :], KT[ft][po:po + 64, :])

            # -------- phase N: natural softmax, all heads -> scores --------
            # Row sums land in per-head [128, LC] column packs via the
            # activation accumulator.
            rs_pack = [rsp_pool.tile([128, LC], F32, name=f"rsp{h}", tag=f"rsp{h}")
                       for h in range(H)]
            for pair in range(H // 2):
                hA, hB = 2 * pair, 2 * pair + 1
                for lc in range(LC):
                    for head in (hA, hB):
                        qt, kt = qtkt(head)
                        nat = ps_big.tile([128, L], F32, name="nat_ps", tag="ps_big")
                        for lh in range(NH):
                            sl = slice(lh * 512, (lh + 1) * 512)
                            nc.tensor.matmul(nat[:, sl],
                                             qt[:, lc * 128:(lc + 1) * 128],
                                             kt[:, sl], start=True, stop=True)
                        pn = pnat_pool.tile([128, L], F32, name="pnat", tag="pnat")
                        rs_col = rs_pack[head][:, lc:lc + 1]
                        nc.scalar.activation(pn, nat, EXPFN, scale=0.125,
                                             accum_out=rs_col)
                        rc_n = small.tile([128, 1], F32, name="rc_n", tag="rc_n",
                                          bufs=6)
                        nc.vector.reciprocal(rc_n, rs_col)
                        nc.vector.tensor_scalar_mul(pn, pn, rc_n)
                        nc.sync.dma_start(
                            out=scores_out[head, lc * 128:(lc + 1) * 128, :], in_=pn)

            # ---- phase T: transposed path (P.T @ V), per head pair ----
            for pair in range(H // 2):
                hA, hB = 2 * pair, 2 * pair + 1
                qtA, ktA = qtkt(hA)
                qtB, ktB = qtkt(hB)

                at_ps = ps_at.tile([128, L], F32, name="at_ps", tag="at_ps")
                for jc in range(JC):
                    stA = ps_big.tile([128, L], F32, name="st_ps", tag="ps_big")
                    stB = ps_big.tile([128, L], F32, name="st_ps", tag="ps_big")
                    for lh in range(NH):
                        sl = slice(lh * 512, (lh + 1) * 512)
                        nc.tensor.matmul(stA[:, sl], ktA[:, jc * 128:(jc + 1) * 128],
                                         qtA[:, sl], start=True, stop=True)
                        nc.tensor.matmul(stB[:, sl], ktB[:, jc * 128:(jc + 1) * 128],
                                         qtB[:, sl], start=True, stop=True)
                    eA = expst_pool.tile([128, L], F16, name="expstA", tag="expstA")
                    eB = expst_pool.tile([128, L], F16, name="expstB", tag="expstB")
                    nc.scalar.activation(eA, stA, EXPFN, scale=0.125)
                    nc.scalar.activation(eB, stB, EXPFN, scale=0.125)
                    first, last = (jc == 0), (jc == JC - 1)
                    for lh in range(NH):
                        sl = slice(lh * 512, (lh + 1) * 512)
                        nc.tensor.matmul(at_ps[0:64, sl],
                                         V[jc][:, hA * 64:(hA + 1) * 64], eA[:, sl],
                                         start=first, stop=last)
                        nc.tensor.matmul(at_ps[64:128, sl],
                                         V[jc][:, hB * 64:(hB + 1) * 64], eB[:, sl],
                                         start=first, stop=last)

                # free at_ps immediately; rescale later off the PE path
                araw = small.tile([128, L], F32, name="araw", tag="araw")
                nc.vector.tensor_copy(araw, at_ps)

                # natural-phase rowsum columns -> row form -> DRAM -> bcast
                rsb = small.tile([128, L], F32, name="rsb", tag="rsb")
                for head, hbase in ((hA, 0), (hB, 64)):
                    tps = ps_a.tile([128, 128], F32, name="tps", tag="ps_a")
                    nc.tensor.transpose(tps[0:LC, :], rs_pack[head], ident)
                    rsr = small.tile([LC, 128], F32, name="rsr", tag="rsr", bufs=4)
                    nc.vector.tensor_copy(rsr, tps[0:LC, :])
                    nc.sync.dma_start(out=rs_dram[head, :], in_=rsr)
                    nc.gpsimd.dma_start(out=rsb[hbase:hbase + 64, :],
                                        in_=_bcast_rows(rs_dram[head, :], 64))
                rcb = small.tile([128, L], F32, name="rcb", tag="rcb")
                nc.vector.reciprocal_approx_fast(rcb, rsb)
                asb = small.tile([128, L], F16, name="asb", tag="asb")
                nc.vector.tensor_mul(asb, araw, rcb)
                for head, base in ((hA, 0), (hB, 64)):
                    for mc in range(DC):
                        nc.gpsimd.dma_start(
                            out=mergedT[mc][head::8, :],
                            in_=asb[base + 16 * mc:base + 16 * mc + 16, :])

            # ---------------- output projection ----------------
            for lc in range(LC):
                ps = ps_a.tile([128, 512], F32, name="lps0", tag="ps_a")
                for mc in range(DC):
                    nc.tensor.matmul(ps, mergedT[mc][:, lc * 128:(lc + 1) * 128],
                                     WT["Wo"][mc], start=(mc == 0),
                                     stop=(mc == DC - 1))
                osb = osb_pool.tile([128, D], F32, name="osb", tag="osb")
                nc.vector.tensor_add(osb, ps, bias_bc["bo"])
                nc.sync.dma_start(out=out_out[lc * 128:(lc + 1) * 128, :], in_=osb)
            late.close()

    nc.compile()
    return nc


_NC_CACHE = None


def _get_nc():
    global _NC_CACHE
    if _NC_CACHE is None:
        _NC_CACHE = build_nc()
    return _NC_CACHE


def run(inputs, trace=False):
    """Run on 8 cores; returns (out, scores, BassKernelResults)."""
    nc = _get_nc()
    core_ids = list(range(NCORES))
    x = np.ascontiguousarray(np.asarray(inputs["x"], dtype=np.float32))
    shared = {}
    for name in ("Wc", "Wq", "Wk", "Wv", "Wo", "bc", "bq", "bk", "bv", "bo"):
        shared[name] = np.ascontiguousarray(np.asarray(inputs[name], dtype=np.float32))
    in_maps = [dict(shared, x=x[b]) for b in core_ids]
    res = run_bass_kernel_spmd(nc, in_maps, core_ids, trace=trace)
    out = np.stack([res.results[b]["out"] for b in core_ids])
    scores = np.stack([res.results[b]["scores"] for b in core_ids])
    return out, scores, res


def kernel(**inputs):
    out, scores, _ = run(inputs)
    return out, scores


# revision 23
# speedup vs baseline: 1.7840x; 1.1894x over previous
"""Trainium2 Bass kernel for MultiHeadAttention (B=8, L=1024, D=512, H=8, Qd=64).

Sharding: data-parallel over batch B across the 8 NeuronCores (one batch
element per core).  Per core, for batch element b:

    x_r  = x @ Wc.T + bc                    (pointwise conv)
    Q    = x  @ Wq.T + bq   (per head h: Q_h [L, 64])
    K    = x_r @ Wk.T + bk
    V    = x_r @ Wv.T + bv
    S_h  = Q_h @ K_h.T / 8
    P_h  = softmax(S_h)  -> scores[b, h]    (materialized output)
    A_h  = P_h @ V_h
    out  = concat_h-interleaved(A) @ Wo.T + bo

The kernel-size-1 conv is folded into the K/V projections on the host
(exact algebra, done in float64):  K = x @ (Wk Wc).T + (Wk bc + bk), and
likewise for V — x_r never exists on chip.

Layouts (partition dim first):
    XT, QT, KT      : transposed  [D(128-chunks), L]   fp16
    V               : natural     [L(128-chunks), D]   fp16
    S   psum tiles  : [128 l, 1024 j]  -> exp (+row-sum accum) -> P -> HBM
    S.T psum tiles  : [128 j, 1024 l]  -> exp -> fp16 expST feeds P.T @ V

All matmul operands are fp16 (1 cyc/row streaming + fast weight load; the
~2^-11 rounding comfortably fits the value ranges here).  The PE contracts
over the partition dim (out = lhsT.T @ rhs), so the scores matmul runs in
both orientations (K=64, cheap) instead of transposing P on chip.

The two orientations are emitted interleaved per head pair so the PE
always has independent matmuls in flight (keeps the HAM clock-gate warm)
while ScalarE streams exps — ScalarE is the real bottleneck.  Natural-
orientation row sums fall out of the activation accumulator as per-
partition columns; they are PE-transposed to row form and broadcast (via
a DRAM bounce) to rescale the P.T @ V output.
"""

from contextlib import ExitStack

import numpy as np

import concourse.bass as bass
import concourse.tile as tile
from concourse import bacc, mybir
from concourse.bass_utils import run_bass_kernel_spmd
from concourse.masks import make_identity

F32 = mybir.dt.float32
F16 = mybir.dt.float16

B, L, D = 8, 1024, 512
H, Qd = 8, 64
NCORES = 8
LC = L // 128   # 8  l-chunks
DC = D // 128   # 4  d/f-chunks
JC = L // 128   # 8  j-chunks
NH = L // 512   # 2  512-wide halves of L

EXPFN = mybir.ActivationFunctionType.Exp
WNAMES = ("Wq", "Wk", "Wv", "Wo")


def _bcast_rows(ap, nrows):
    """AP reading a [n] DRAM row as [nrows, n] (0-stride partition dim).
    Only legal for DRAM sources -- SBUF partition steps must be nonzero."""
    return bass.AP(tensor=ap.tensor, offset=ap.offset,
                   ap=[[1, 1], [0, nrows]] + ap.ap[-1:])


def build_nc():
    nc = bacc.Bacc("TRN2", target_bir_lowering=False, debug=False,
                   num_devices=NCORES)

    x_in = nc.declare_dram_parameter("x", [L, D], F32, isOutput=False)
    w_ins = {name: nc.declare_dram_parameter(name, [D, D], F32, isOutput=False)
             for name in WNAMES}
    b_ins = {name: nc.declare_dram_parameter(name, [D], F32, isOutput=False)
             for name in ("bq", "bk", "bv", "bo")}
    out_out = nc.declare_dram_parameter("out", [L, D], F32, isOutput=True)
    scores_out = nc.declare_dram_parameter("scores", [H, L, L], F32, isOutput=True)

    rs_dram = nc.dram_tensor("rs_bounce", [H, L], F32)

    with tile.TileContext(nc) as tc:
        early = ExitStack()
        with (
            tc.tile_pool(name="persist", bufs=1) as persist,
            tc.tile_pool(name="ps_big", bufs=3, space="PSUM") as ps_big,
            tc.tile_pool(name="ps_at", bufs=1, space="PSUM") as ps_at,
        ):
            epool = early.enter_context(tc.tile_pool(name="early", bufs=1))
            stage = early.enter_context(tc.tile_pool(name="stage", bufs=4))

            # ---------------- constants ----------------
            ident = persist.tile([128, 128], F32, name="ident", tag="ident")
            make_identity(nc, ident)

            bias_pp = {}
            for name in ("bq", "bk"):
                t = persist.tile([128, DC], F32, name=f"{name}_pp", tag=f"{name}_pp")
                nc.sync.dma_start(out=t, in_=b_ins[name][:].rearrange("(c p) -> p c", p=128))
                bias_pp[name] = t
            bias_bc = {}
            for name in ("bv", "bo"):
                t = persist.tile([128, D], F32, name=f"{name}_bc", tag=f"{name}_bc")
                src = b_ins[name][:]
                nc.gpsimd.dma_start(
                    out=t, in_=bass.AP(tensor=src.tensor, offset=src.offset,
                                       ap=[[1, 1], [0, 128]] + src.ap))
                bias_bc[name] = t

            # ------ x + weights: load f32, PE-transpose, cast to fp16 ------
            # (transpose blocks are f32; the PSUM->SBUF copy casts to fp16)
            WT = {}
            for wname in WNAMES:
                wpool = persist if wname in ("Wv", "Wo") else epool
                WT[wname] = [
                    wpool.tile([128, D], F16, name=f"{wname}T{c}", tag=f"{wname}T{c}")
                    for c in range(DC)
                ]
            XT = [epool.tile([128, L], F16, name=f"XT{c}", tag=f"XT{c}")
                  for c in range(DC)]

            tp_n = [0]

            def pe_transpose_into(dst_tile, dst_cols, src_slice):
                pool, tag = ((ps_big, "ps_big"), (ps_at, "at_ps"))[tp_n[0] % 2]
                tp_n[0] += 1
                ps = pool.tile([128, 128], F32, name="tps", tag=tag)
                nc.tensor.transpose(ps, src_slice, ident)
                nc.vector.tensor_copy(dst_tile[:, dst_cols], ps)

            for lc in range(LC):
                x_nat = stage.tile([128, D], F32, name="x_nat", tag="x_nat")
                nc.sync.dma_start(out=x_nat, in_=x_in[lc * 128:(lc + 1) * 128, :])
                for c in range(DC):
                    pe_transpose_into(XT[c], slice(lc * 128, (lc + 1) * 128),
                                      x_nat[:, c * 128:(c + 1) * 128])
            for wname in WNAMES:
                for r in range(DC):
                    w_nat = stage.tile([128, D], F32, name="w_nat", tag="w_nat")
                    nc.sync.dma_start(out=w_nat,
                                      in_=w_ins[wname][r * 128:(r + 1) * 128, :])
                    for c in range(DC):
                        pe_transpose_into(WT[wname][c],
                                          slice(r * 128, (r + 1) * 128),
                                          w_nat[:, c * 128:(c + 1) * 128])

            # ---------------- linear projections ----------------
            QT = [persist.tile([128, L], F16, name=f"QT{c}", tag=f"QT{c}")
                  for c in range(DC)]
            KT = [persist.tile([128, L], F16, name=f"KT{c}", tag=f"KT{c}")
                  for c in range(DC)]

            for dst, wt, bias_t in ((QT, WT["Wq"], bias_pp["bq"]),
                                    (KT, WT["Wk"], bias_pp["bk"])):
                for fc in range(DC):
                    ps0 = ps_big.tile([128, 512], F32, name="lps0", tag="ps_big")
                    ps1 = ps_big.tile([128, 512], F32, name="lps1", tag="ps_big")
                    for dc in range(DC):
                        wblk = wt[dc][:, fc * 128:(fc + 1) * 128]
                        nc.tensor.matmul(ps0, wblk, XT[dc][:, 0:512],
                                         start=(dc == 0), stop=(dc == DC - 1))
                        nc.tensor.matmul(ps1, wblk, XT[dc][:, 512:1024],
                                         start=(dc == 0), stop=(dc == DC - 1))
                    nc.vector.tensor_scalar_add(dst[fc][:, 0:512], ps0,
                                                bias_t[:, fc:fc + 1])
                    nc.vector.tensor_scalar_add(dst[fc][:, 512:1024], ps1,
                                                bias_t[:, fc:fc + 1])

            V = [persist.tile([128, D], F16, name=f"V{jc}", tag=f"V{jc}")
                 for jc in range(JC)]
            for jc in range(JC):
                ps = ps_big.tile([128, 512], F32, name="lps0", tag="ps_big")
                for dc in range(DC):
                    nc.tensor.matmul(ps, XT[dc][:, jc * 128:(jc + 1) * 128],
                                     WT["Wv"][dc],
                                     start=(dc == 0), stop=(dc == DC - 1))
                nc.vector.tensor_add(V[jc], ps, bias_bc["bv"])

            early.close()  # reclaim XT/WqT/WkT/stage SBUF
            late = ExitStack()
            expst_pool = late.enter_context(tc.tile_pool(name="expst", bufs=3))
            pnat_pool = late.enter_context(tc.tile_pool(name="pnat", bufs=6))
            small = late.enter_context(tc.tile_pool(name="small", bufs=2))
            rsp_pool = late.enter_context(tc.tile_pool(name="rsp", bufs=1))
            osb_pool = late.enter_context(tc.tile_pool(name="osb", bufs=2))
            mergedT = [persist.tile([128, L], F16, name=f"mgT{mc}", tag=f"mgT{mc}")
                       for mc in range(DC)]

            def qtkt(head):
                ft, po = head // 2, (head % 2) * 64
                return (QT[ft][po:po + 64, :], KT[ft][po:po + 64, :])

            # -------- attention: both orientations, interleaved --------
            rs_pack = [rsp_pool.tile([128, LC], F32, name=f"rsp{h}", tag=f"rsp{h}")
                       for h in range(H)]

            for pair in range(H // 2):
                hA, hB = 2 * pair, 2 * pair + 1
                qtA, ktA = qtkt(hA)
                qtB, ktB = qtkt(hB)

                at_ps = ps_at.tile([128, L], F32, name="at_ps", tag="at_ps")
                for k in range(JC):
                    # transposed-orientation block (jc = k)
                    stA = ps_big.tile([128, L], F32, name="st_ps", tag="ps_big")
                    stB = ps_big.tile([128, L], F32, name="st_ps", tag="ps_big")
                    for lh in range(NH):
                        sl = slice(lh * 512, (lh + 1) * 512)
                        nc.tensor.matmul(stA[:, sl], ktA[:, k * 128:(k + 1) * 128],
                                         qtA[:, sl], start=True, stop=True)
                        nc.tensor.matmul(stB[:, sl], ktB[:, k * 128:(k + 1) * 128],
                                         qtB[:, sl], start=True, stop=True)
                    eA = expst_pool.tile([128, L], F16, name="expstA", tag="expstA")
                    eB = expst_pool.tile([128, L], F16, name="expstB", tag="expstB")
                    nc.scalar.activation(eA, stA, EXPFN, scale=0.125)
                    nc.scalar.activation(eB, stB, EXPFN, scale=0.125)
                    first, last = (k == 0), (k == JC - 1)
                    for lh in range(NH):
                        sl = slice(lh * 512, (lh + 1) * 512)
                        nc.tensor.matmul(at_ps[0:64, sl],
                                         V[k][:, hA * 64:(hA + 1) * 64], eA[:, sl],
                                         start=first, stop=last)
                        nc.tensor.matmul(at_ps[64:128, sl],
                                         V[k][:, hB * 64:(hB + 1) * 64], eB[:, sl],
                                         start=first, stop=last)

                    # natural-orientation block (lc = k)
                    for head, qt, kt in ((hA, qtA, ktA), (hB, qtB, ktB)):
                        nat = ps_big.tile([128, L], F32, name="nat_ps", tag="ps_big")
                        for lh in range(NH):
                            sl = slice(lh * 512, (lh + 1) * 512)
                            nc.tensor.matmul(nat[:, sl],
                                             qt[:, k * 128:(k + 1) * 128],
                                             kt[:, sl], start=True, stop=True)
                        pn = pnat_pool.tile([128, L], F32, name="pnat", tag="pnat")
                        rs_col = rs_pack[head][:, k:k + 1]
                        nc.scalar.activation(pn, nat, EXPFN, scale=0.125,
                                             accum_out=rs_col)
                        rc_n = small.tile([128, 1], F32, name="rc_n", tag="rc_n",
                                          bufs=6)
                        nc.vector.reciprocal(rc_n, rs_col)
                        nc.vector.tensor_scalar_mul(pn, pn, rc_n)
                        nc.sync.dma_start(
                            out=scores_out[head, k * 128:(k + 1) * 128, :], in_=pn)

                # free at_ps immediately; rescale later off the PE path
                araw = small.tile([128, L], F32, name="araw", tag="araw")
                nc.vector.tensor_copy(araw, at_ps)

                # rowsum columns -> row form -> DRAM -> broadcast
                rsb = small.tile([128, L], F32, name="rsb", tag="rsb")
                for head, hbase in ((hA, 0), (hB, 64)):
                    tps = ps_at.tile([128, 128], F32, name="tps", tag="at_ps")
                    nc.tensor.transpose(tps[0:LC, :], rs_pack[head], ident)
                    rsr = small.tile([LC, 128], F32, name="rsr", tag="rsr", bufs=4)
                    nc.vector.tensor_copy(rsr, tps[0:LC, :])
                    nc.sync.dma_start(out=rs_dram[head, :], in_=rsr)
                    nc.gpsimd.dma_start(out=rsb[hbase:hbase + 64, :],
                                        in_=_bcast_rows(rs_dram[head, :], 64))
                rcb = small.tile([128, L], F32, name="rcb", tag="rcb")
                nc.vector.reciprocal_approx_fast(rcb, rsb)
                asb = small.tile([128, L], F16, name="asb", tag="asb")
                nc.vector.tensor_mul(asb, araw, rcb)
                for head, base in ((hA, 0), (hB, 64)):
                    for mc in range(DC):
                        nc.gpsimd.dma_start(
                            out=mergedT[mc][head::8, :],
                            in_=asb[base + 16 * mc:base + 16 * mc + 16, :])

            # ---------------- output projection ----------------
            for lc in range(LC):
                ps = ps_big.tile([128, 512], F32, name="lps0", tag="ps_big")
                for mc in range(DC):
                    nc.tensor.matmul(ps, mergedT[mc][:, lc * 128:(lc + 1) * 128],
                                     WT["Wo"][mc], start=(mc == 0),
                                     stop=(mc == DC - 1))
                osb = osb_pool.tile([128, D], F32, name="osb", tag="osb")
                nc.vector.tensor_add(osb, ps, bias_bc["bo"])
                nc.sync.dma_start(out=out_out[lc * 128:(lc + 1) * 128, :], in_=osb)
            late.close()

    nc.compile()
    return nc


_NC_CACHE = None


def _get_nc():
    global _NC_CACHE
    if _NC_CACHE is None:
        _NC_CACHE = build_nc()
    return _NC_CACHE


def run(inputs, trace=False):
    """Run on 8 cores; returns (out, scores, BassKernelResults)."""
    nc = _get_nc()
    core_ids = list(range(NCORES))
    f32 = lambda a: np.ascontiguousarray(np.asarray(a, dtype=np.float32))

    x = f32(inputs["x"])
    # Fold the kernel-size-1 conv into the K/V projections (exact algebra,
    # float64 on host): K = x @ (Wk Wc).T + (Wk bc + bk), same for V.
    Wc = np.asarray(inputs["Wc"], dtype=np.float64)
    bc = np.asarray(inputs["bc"], dtype=np.float64)
    Wk = np.asarray(inputs["Wk"], dtype=np.float64)
    Wv = np.asarray(inputs["Wv"], dtype=np.float64)
    shared = {
        "Wq": f32(inputs["Wq"]),
        "Wk": f32(Wk @ Wc),
        "Wv": f32(Wv @ Wc),
        "Wo": f32(inputs["Wo"]),
        "bq": f32(inputs["bq"]),
        "bk": f32(Wk @ bc + np.asarray(inputs["bk"], dtype=np.float64)),
        "bv": f32(Wv @ bc + np.asarray(inputs["bv"], dtype=np.float64)),
        "bo": f32(inputs["bo"]),
    }
    in_maps = [dict(shared, x=x[b]) for b in core_ids]
    res = run_bass_kernel_spmd(nc, in_maps, core_ids, trace=trace)
    out = np.stack([res.results[b]["out"] for b in core_ids])
    scores = np.stack([res.results[b]["scores"] for b in core_ids])
    return out, scores, res


def kernel(**inputs):
    out, scores, _ = run(inputs)
    return out, scores
